# revision 11
# baseline (speedup 1.0000x reference)
"""Trainium2 Bass kernel for nn_Attention_16801912062520.

Reference computation (jax):
    S4   = S.reshape(dps, seq, H, DK)
    S_Q  = S4 @ WQ_w.T + WQ_b
    R_K  = R4 @ WK_w.T + WK_b
    R_V  = R4 @ WV_w.T + WV_b
    beta = sum(S_Q * R_K, -1)
    out  = where(S_mas, R_V * beta, 0)

Algebraic reduction (exact): beta[b,s,h] = S[b,s,:] . qv[b,h,:] + c[b,h]
with qv[b,h,:] = WQ_w.T @ R_K[b,h,:] embedded in head h's 64-wide slice of d,
and c[b,h] = WQ_b . R_K[b,h,:].  The big projection einsum never needs to be
materialized; the kernel is memory-bound (read S + write out).

Sharding: batch (dps=32) split 4-per-core across 8 cores; tiny per-batch
vectors (qv, R_V, c, mask) are precomputed on host and shipped per core.

Device kernel per 512-row super-tile:
  DMA in [128,4,1024] -> PE transpose (8 chunks x 4 subtiles) -> ACT copy
  PSUM->SBUF -> 8 accumulating matmuls (K=128 each) -> beta^T [16,512] ->
  ACT bias add -> 8 expand matmuls (beta^T x Vexp block-diag) -> DVE masked
  PSUM->SBUF copy -> DMA out.
"""

import os
import numpy as np

H, DK = 16, 64
DPS, SEQ, D = 32, 2048, 1024
NCORES = 8
NB = DPS // NCORES          # batches per core
SUP = 512                   # seq rows per super-tile
NSUP = SEQ // SUP           # super-tiles per batch
NSUB = SUP // 128           # 128-row subtiles per super-tile

# float32r: single-pass fp32 matmul (full PE rate at N>=512).  Falls back to
# exact 2-pass fp32 matmuls when False.
USE_F32R = True

_CACHE = {}


def _build_nc(nb=NB, use_f32r=USE_F32R):
    import concourse.bacc as bacc
    import concourse.mybir as mybir
    from concourse import masks
    from concourse.tile import TileContext
    from contextlib import ExitStack

    f32 = mybir.dt.float32
    # fp32r: single-pass fp32 matmul. The BIR verifier requires every tensor
    # consumed by an fp32r matmul to be fp32r-typed at its producer.
    fmm = mybir.dt.float32r if use_f32r else f32

    nc = bacc.Bacc("TRN2", target_bir_lowering=False, debug=False)

    S = nc.dram_tensor("S", [nb, SEQ, D], fmm, kind="ExternalInput")
    qvTh = nc.dram_tensor("qvTh", [128, nb * 8 * 16], fmm, kind="ExternalInput")
    vexph = nc.dram_tensor("vexph", [16, nb * D], fmm, kind="ExternalInput")
    cvech = nc.dram_tensor("cvech", [16, nb], f32, kind="ExternalInput")
    maskh = nc.dram_tensor("maskh", [128, nb * 16], f32, kind="ExternalInput")
    out = nc.dram_tensor("out", [nb, SEQ, D], f32, kind="ExternalOutput")

    with TileContext(nc) as tc, ExitStack() as ctx:
        consts = ctx.enter_context(tc.tile_pool(name="consts", bufs=1))
        sin_pool = ctx.enter_context(tc.tile_pool(name="sin", bufs=3))
        st_pool = ctx.enter_context(tc.tile_pool(name="st", bufs=2))
        osb_pool = ctx.enter_context(tc.tile_pool(name="osb", bufs=2))
        bsb_pool = ctx.enter_context(tc.tile_pool(name="bsb", bufs=2))
        psT_pool = ctx.enter_context(tc.tile_pool(name="psT", bufs=2, space="PSUM"))
        bps_pool = ctx.enter_context(tc.tile_pool(name="bps", bufs=1, space="PSUM"))
        ops_pool = ctx.enter_context(tc.tile_pool(name="ops", bufs=2, space="PSUM"))
        heat_pool = ctx.enter_context(tc.tile_pool(name="heat", bufs=1, space="PSUM"))

        # first super-tile load goes ahead of the const loads (same HWDGE FIFO)
        s_srcs = [S[b].rearrange("(u j p) d -> u p j d", u=NSUP, j=NSUB, p=128)
                  for b in range(nb)]
        o_dsts = [out[b].rearrange("(u j p) d -> u p j d", u=NSUP, j=NSUB, p=128)
                  for b in range(nb)]
        s_sup0 = sin_pool.tile([128, NSUB, D], fmm, tag="s_sup")
        nc.sync.dma_start(s_sup0[:], s_srcs[0][0])

        ident_f32 = consts.tile([128, 128], f32)
        masks.make_identity(nc, ident_f32[:])
        if use_f32r:
            ident_r = consts.tile([128, 128], fmm)
            nc.scalar.copy(ident_r[:], ident_f32[:])
            ident = ident_r[:]
        else:
            ident = ident_f32[:]
        # HAM heater: transpose-mode does not count as PE activity for the
        # clock gate, so a long transpose stretch lets the PE re-throttle to
        # 1.2 GHz.  A tiny real matmul every couple of microseconds keeps the
        # MID window from ever seeing a fully idle 3.4us -> PE stays warm.
        heat_c = consts.tile([1, 65], f32)
        nc.vector.memset(heat_c[:], 0.0)
        qvT_sb = consts.tile([128, nb * 8 * 16], fmm)
        nc.sync.dma_start(qvT_sb[:], qvTh[:, :])
        vexp_sb = consts.tile([16, nb * D], fmm)
        nc.sync.dma_start(vexp_sb[:], vexph[:, :])
        cvec_sb = consts.tile([16, nb], f32)
        nc.sync.dma_start(cvec_sb[:], cvech[:, :])
        mask_sb = consts.tile([128, nb * 16], f32)
        nc.sync.dma_start(mask_sb[:], maskh[:, :])

        heat_ps = heat_pool.tile([1, 64], f32)
        for b in range(nb):
            for u in range(NSUP):
                if b == 0 and u == 0:
                    s_sup = s_sup0
                else:
                    s_sup = sin_pool.tile([128, NSUB, D], fmm, tag="s_sup")
                    nc.sync.dma_start(s_sup[:], s_srcs[b][u])

                # S^T staging: sT[dl, cg, 128j + p] = S[b, 512u+128j+p, 128cg+dl]
                # Emitted in 8 groups of 4 transposes with the beta matmuls
                # interspersed so the PE's HAM activity monitor never sees a
                # >3.4us stretch without a real matmul (transpose-mode does
                # not count as PE-busy for HAM).
                sT = st_pool.tile([128, 8, SUP], fmm)
                bps = bps_pool.tile([16, SUP], f32)
                for g4 in range(4):                 # chunk pair (2*g4, 2*g4+1)
                    for jp in range(2):             # subtile pair (2*jp, 2*jp+1)
                        nc.tensor.matmul(
                            heat_ps[:], heat_c[:, 0:1], heat_c[:, 1:65],
                            start=True, stop=True, skip_group_check=True,
                        )
                        psT = psT_pool.tile([128, 2, 2, 128], fmm)
                        for jj in range(2):
                            for cc in range(2):
                                cg = 2 * g4 + cc
                                j = 2 * jp + jj
                                nc.tensor.transpose(
                                    psT[:, jj, cc, :],
                                    s_sup[:, j, 128 * cg:128 * (cg + 1)],
                                    ident,
                                )
                        nc.scalar.copy(
                            sT[:, 2 * g4:2 * (g4 + 1), 256 * jp:256 * (jp + 1)]
                            .rearrange("p c (j s) -> p c j s", j=2),
                            psT[:].rearrange("p j c s -> p c j s"),
                        )
                    for cc in range(2):
                        cg = 2 * g4 + cc
                        lhsT = qvT_sb[:, (b * 8 + cg) * 16:(b * 8 + cg + 1) * 16]
                        nc.tensor.matmul(
                            bps[:], lhsT, sT[:, cg, :],
                            start=(cg == 0), stop=(cg == 7),
                        )
                bsb = bsb_pool.tile([16, SUP], fmm)
                nc.scalar.add(bsb[:], bps[:], cvec_sb[:, b:b + 1])

                # expand: out[p, f] = beta2[h(f), s'] * Vexp[h(f), f], then mask
                o_sup = osb_pool.tile([128, NSUB, D], f32)
                for j in range(NSUB):
                    ops = ops_pool.tile([128, D], f32)
                    lhsT = bsb[:, 128 * j:128 * (j + 1)]
                    for half in range(2):
                        rhs = vexp_sb[:, b * D + 512 * half:b * D + 512 * (half + 1)]
                        nc.tensor.matmul(
                            ops[:, 512 * half:512 * (half + 1)],
                            lhsT, rhs,
                            start=True, stop=True,
                        )
                    t = u * NSUB + j
                    nc.vector.tensor_scalar_mul(
                        o_sup[:, j, :], ops[:],
                        mask_sb[:, b * 16 + t:b * 16 + t + 1],
                    )
                    if j % 2 == 1:  # store in halves: shorter kernel tail
                        nc.sync.dma_start(
                            o_dsts[b][u, :, j - 1:j + 1, :],
                            o_sup[:, j - 1:j + 1, :],
                        )

    nc.compile()
    return nc


def _host_prep(S, R, S_mas, WQ_w, WQ_b, WK_w, WK_b, WV_w, WV_b):
    """Tiny per-(batch, head) vectors derived from R and the dk x dk weights."""
    R4 = np.asarray(R, np.float32).reshape(DPS, H, DK)
    R_K = np.einsum("bhd,ed->bhe", R4, np.asarray(WK_w, np.float32)) + np.asarray(WK_b, np.float32)
    R_V = np.einsum("bhd,ed->bhe", R4, np.asarray(WV_w, np.float32)) + np.asarray(WV_b, np.float32)
    qv = np.einsum("ed,bhe->bhd", np.asarray(WQ_w, np.float32), R_K)      # (dps, H, DK)
    c = R_K @ np.asarray(WQ_b, np.float32)                                 # (dps, H)
    maskf = (np.asarray(S_mas).reshape(DPS, SEQ) != 0).astype(np.float32)

    in_maps = []
    for k in range(NCORES):
        sl = slice(k * NB, (k + 1) * NB)
        qv_c, rv_c, c_c, m_c = qv[sl], R_V[sl], c[sl], maskf[sl]

        qvT_packed = np.zeros((NB, 8, 128, 16), np.float32)
        for h in range(H):
            cg, j = divmod(h, 2)
            qvT_packed[:, cg, 64 * j:64 * (j + 1), h] = qv_c[:, h, :]
        qvTh = np.ascontiguousarray(
            qvT_packed.transpose(2, 0, 1, 3).reshape(128, NB * 8 * 16))

        vexp = np.zeros((NB, H, D), np.float32)
        for h in range(H):
            vexp[:, h, 64 * h:64 * (h + 1)] = rv_c[:, h, :]
        vexph = np.ascontiguousarray(vexp.transpose(1, 0, 2).reshape(16, NB * D))

        cvech = np.ascontiguousarray(c_c.T)                                # (16, nb)
        maskh = np.ascontiguousarray(
            m_c.reshape(NB, 16, 128).transpose(2, 0, 1).reshape(128, NB * 16))

        in_maps.append({
            "S": np.ascontiguousarray(np.asarray(S, np.float32)[sl]),
            "qvTh": qvTh,
            "vexph": vexph,
            "cvech": cvech,
            "maskh": maskh,
        })
    return in_maps


def kernel(S, R, S_mas, R_mas, WQ_w, WQ_b, WK_w, WK_b, WV_w, WV_b):
    from concourse.bass_utils import run_bass_kernel_spmd

    in_maps = _host_prep(S, R, S_mas, WQ_w, WQ_b, WK_w, WK_b, WV_w, WV_b)

    if "nc" not in _CACHE:
        _CACHE["nc"] = _build_nc()
    nc = _CACHE["nc"]

    res = run_bass_kernel_spmd(nc, in_maps, core_ids=list(range(NCORES)))
    out = np.concatenate([r["out"] for r in res.results], axis=0)
    return out.astype(np.float32)


# revision 12
# speedup vs baseline: 1.3974x; 1.3974x over previous
"""Trainium2 Bass kernel for nn_Attention_16801912062520.

Reference computation (jax):
    S4   = S.reshape(dps, seq, H, DK)
    S_Q  = S4 @ WQ_w.T + WQ_b
    R_K  = R4 @ WK_w.T + WK_b
    R_V  = R4 @ WV_w.T + WV_b
    beta = sum(S_Q * R_K, -1)
    out  = where(S_mas, R_V * beta, 0)

Algebraic reduction (exact): beta[b,s,h] = S[b,s,:] . qv[b,h,:] + c[b,h]
with qv[b,h,:] = WQ_w.T @ R_K[b,h,:] embedded in head h's 64-wide slice of d,
and c[b,h] = WQ_b . R_K[b,h,:].  The big projection einsum never needs to be
materialized; the kernel is memory-bound (read S + write out).

Sharding: batch (dps=32) split 4-per-core across 8 cores; tiny per-batch
vectors (qv, R_V, c, mask) are precomputed on host and shipped per core.

Device kernel per 512-row super-tile (input DMA'd 2 supers at a time, 4MB):
  PE transpose (8 chunks x 4 subtiles) -> ACT copy PSUM->SBUF -> 8
  accumulating f32r matmuls -> beta^T [16,512] -> ACT bias add -> 8 expand
  f32r matmuls (beta^T x Vexp block-diag) -> DVE masked PSUM->SBUF copy ->
  2MB DMA out.
"""

import numpy as np

H, DK = 16, 64
DPS, SEQ, D = 32, 2048, 1024
NCORES = 8
NB = DPS // NCORES          # batches per core
SUP = 512                   # seq rows per super-tile (one compute round)
NSUP = SEQ // SUP           # super-tiles per batch
NSUB = SUP // 128           # 128-row subtiles per super-tile
NDBL = SEQ // (2 * SUP)     # double-supers (DMA granularity) per batch

# float32r: single-pass fp32 matmul (full PE rate at N>=512).  Falls back to
# exact 2-pass fp32 matmuls when False.
USE_F32R = True

_CACHE = {}


def _build_nc(nb=NB, use_f32r=USE_F32R):
    import concourse.bacc as bacc
    import concourse.mybir as mybir
    from concourse import masks
    from concourse.tile import TileContext
    from contextlib import ExitStack

    f32 = mybir.dt.float32
    # fp32r: single-pass fp32 matmul. The BIR verifier requires every tensor
    # consumed by an fp32r matmul to be fp32r-typed at its producer.
    fmm = mybir.dt.float32r if use_f32r else f32

    nc = bacc.Bacc("TRN2", target_bir_lowering=False, debug=False)

    S = nc.dram_tensor("S", [nb, SEQ, D], f32, kind="ExternalInput")
    qvTh = nc.dram_tensor("qvTh", [128, nb * 8 * 16], fmm, kind="ExternalInput")
    vexph = nc.dram_tensor("vexph", [16, nb * D], fmm, kind="ExternalInput")
    cvech = nc.dram_tensor("cvech", [16, nb], f32, kind="ExternalInput")
    maskh = nc.dram_tensor("maskh", [128, nb * 16], f32, kind="ExternalInput")
    out = nc.dram_tensor("out", [nb, SEQ, D], f32, kind="ExternalOutput")

    with TileContext(nc) as tc, ExitStack() as ctx:
        consts = ctx.enter_context(tc.tile_pool(name="consts", bufs=1))
        sin_pool = ctx.enter_context(tc.tile_pool(name="sin", bufs=2))
        st_pool = ctx.enter_context(tc.tile_pool(name="st", bufs=2))
        osb_pool = ctx.enter_context(tc.tile_pool(name="osb", bufs=2))
        bsb_pool = ctx.enter_context(tc.tile_pool(name="bsb", bufs=2))
        psT_pool = ctx.enter_context(tc.tile_pool(name="psT", bufs=2, space="PSUM"))
        bps_pool = ctx.enter_context(tc.tile_pool(name="bps", bufs=2, space="PSUM"))
        ops_pool = ctx.enter_context(tc.tile_pool(name="ops", bufs=2, space="PSUM"))

        # Small const loads first (they head the HWDGE FIFO but cost ~1us and
        # unblock the PE warm-up clump below).
        ident_f32 = consts.tile([128, 128], f32)
        masks.make_identity(nc, ident_f32[:])
        qvT_sb = consts.tile([128, nb * 8 * 16], fmm)
        nc.sync.dma_start(qvT_sb[:], qvTh[:, :])
        vexp_sb = consts.tile([16, nb * D], fmm)
        nc.sync.dma_start(vexp_sb[:], vexph[:, :])
        cvec_sb = consts.tile([16, nb], f32)
        nc.sync.dma_start(cvec_sb[:], cvech[:, :])
        mask_sb = consts.tile([128, nb * 16], f32)
        nc.sync.dma_start(mask_sb[:], maskh[:, :])

        # Rows of S/out viewed as (ud, j, p): s = 1024*ud + 128*j + p
        s_srcs = [S[b].rearrange("(ud j p) d -> ud p j d", ud=NDBL, j=2 * NSUB, p=128)
                  for b in range(nb)]
        o_dsts = [out[b].rearrange("(ud j p) d -> ud p j d", ud=NDBL, j=2 * NSUB, p=128)
                  for b in range(nb)]
        s_dbl0 = sin_pool.tile([128, 2 * NSUB, D], f32, tag="s_dbl")
        nc.sync.dma_start(s_dbl0[:], s_srcs[0][0])

        # Warm-up clump: ~24 back-to-back real matmuls running under the
        # first 4MB input DMA lift the PE HAM clock gate to 2.4 GHz before
        # the first super-tile computes.  Results are discarded.
        warm_ps = bps_pool.tile([16, SUP], f32, tag="bps")
        for _ in range(24):
            nc.tensor.matmul(warm_ps[:], qvT_sb[:, 0:16], qvT_sb[:, 0:SUP],
                             start=True, stop=True)

        for b in range(nb):
            for ud in range(NDBL):
                if b == 0 and ud == 0:
                    s_dbl = s_dbl0
                else:
                    s_dbl = sin_pool.tile([128, 2 * NSUB, D], f32, tag="s_dbl")
                    nc.sync.dma_start(s_dbl[:], s_srcs[b][ud])

                for half in range(2):
                    u = 2 * ud + half

                    # S^T staging:
                    #   sT[dl, cg, 128j+p] = S[b, 512u+128j+p, 128cg+dl]
                    sT = st_pool.tile([128, 8, SUP], fmm)
                    for j in range(NSUB):
                        jg = NSUB * half + j
                        for g in range(2):
                            psT = psT_pool.tile([128, 4, 128], f32)
                            for ci in range(4):
                                cg = 4 * g + ci
                                nc.tensor.transpose(
                                    psT[:, ci, :],
                                    s_dbl[:, jg, 128 * cg:128 * (cg + 1)],
                                    ident_f32[:],
                                )
                            nc.scalar.copy(
                                sT[:, 4 * g:4 * (g + 1), 128 * j:128 * (j + 1)],
                                psT[:],
                            )

                    # beta^T[h, s'] accumulated over the 8 d-chunks
                    bps = bps_pool.tile([16, SUP], f32, tag="bps")
                    for cg in range(8):
                        lhsT = qvT_sb[:, (b * 8 + cg) * 16:(b * 8 + cg + 1) * 16]
                        nc.tensor.matmul(
                            bps[:], lhsT, sT[:, cg, :],
                            start=(cg == 0), stop=(cg == 7),
                        )
                    bsb = bsb_pool.tile([16, SUP], fmm)
                    nc.scalar.add(bsb[:], bps[:], cvec_sb[:, b:b + 1])

                    # expand + mask: out[p, f] = beta2[h(f), s'] * Vexp[h(f), f]
                    o_sup = osb_pool.tile([128, NSUB, D], f32)
                    for j in range(NSUB):
                        ops = ops_pool.tile([128, D], f32)
                        lhsT = bsb[:, 128 * j:128 * (j + 1)]
                        for hf in range(2):
                            rhs = vexp_sb[:, b * D + 512 * hf:b * D + 512 * (hf + 1)]
                            nc.tensor.matmul(
                                ops[:, 512 * hf:512 * (hf + 1)],
                                lhsT, rhs,
                                start=True, stop=True,
                            )
                        t = u * NSUB + j
                        nc.vector.tensor_scalar_mul(
                            o_sup[:, j, :], ops[:],
                            mask_sb[:, b * 16 + t:b * 16 + t + 1],
                        )
                    nc.sync.dma_start(
                        o_dsts[b][ud, :, NSUB * half:NSUB * (half + 1), :],
                        o_sup[:],
                    )

    nc.compile()
    return nc


def _host_prep(S, R, S_mas, WQ_w, WQ_b, WK_w, WK_b, WV_w, WV_b):
    """Tiny per-(batch, head) vectors derived from R and the dk x dk weights."""
    R4 = np.asarray(R, np.float32).reshape(DPS, H, DK)
    R_K = np.einsum("bhd,ed->bhe", R4, np.asarray(WK_w, np.float32)) + np.asarray(WK_b, np.float32)
    R_V = np.einsum("bhd,ed->bhe", R4, np.asarray(WV_w, np.float32)) + np.asarray(WV_b, np.float32)
    qv = np.einsum("ed,bhe->bhd", np.asarray(WQ_w, np.float32), R_K)      # (dps, H, DK)
    c = R_K @ np.asarray(WQ_b, np.float32)                                 # (dps, H)
    maskf = (np.asarray(S_mas).reshape(DPS, SEQ) != 0).astype(np.float32)

    in_maps = []
    for k in range(NCORES):
        sl = slice(k * NB, (k + 1) * NB)
        qv_c, rv_c, c_c, m_c = qv[sl], R_V[sl], c[sl], maskf[sl]

        qvT_packed = np.zeros((NB, 8, 128, 16), np.float32)
        for h in range(H):
            cg, j = divmod(h, 2)
            qvT_packed[:, cg, 64 * j:64 * (j + 1), h] = qv_c[:, h, :]
        qvTh = np.ascontiguousarray(
            qvT_packed.transpose(2, 0, 1, 3).reshape(128, NB * 8 * 16))

        vexp = np.zeros((NB, H, D), np.float32)
        for h in range(H):
            vexp[:, h, 64 * h:64 * (h + 1)] = rv_c[:, h, :]
        vexph = np.ascontiguousarray(vexp.transpose(1, 0, 2).reshape(16, NB * D))

        cvech = np.ascontiguousarray(c_c.T)                                # (16, nb)
        maskh = np.ascontiguousarray(
            m_c.reshape(NB, 16, 128).transpose(2, 0, 1).reshape(128, NB * 16))

        in_maps.append({
            "S": np.ascontiguousarray(np.asarray(S, np.float32)[sl]),
            "qvTh": qvTh,
            "vexph": vexph,
            "cvech": cvech,
            "maskh": maskh,
        })
    return in_maps


def kernel(S, R, S_mas, R_mas, WQ_w, WQ_b, WK_w, WK_b, WV_w, WV_b):
    from concourse.bass_utils import run_bass_kernel_spmd

    in_maps = _host_prep(S, R, S_mas, WQ_w, WQ_b, WK_w, WK_b, WV_w, WV_b)

    if "nc" not in _CACHE:
        _CACHE["nc"] = _build_nc()
    nc = _CACHE["nc"]

    res = run_bass_kernel_spmd(nc, in_maps, core_ids=list(range(NCORES)))
    out = np.concatenate([r["out"] for r in res.results], axis=0)
    return out.astype(np.float32)
